# revision 36
# baseline (speedup 1.0000x reference)
"""Trainium2 Bass kernel for nn_Attention_79207786873625.

Non-local attention block: 1x1 convs (theta/phi/g) -> maxpool2x2(phi,g) ->
scores = theta^T phi -> softmax over m -> o = g beta^T -> w_o conv ->
gamma*o + x.   Shapes: B=16, C=256, H=W=64 (n=HW=4096, m=HW/4=1024).

Sharding: data-parallel over batch across 8 cores (2 samples/core),
weights replicated, per-sample score matrix device-local.

Design:
- Scores are computed TRANSPOSED, sT[m, n] (m on partitions), so the
  attend contraction (over m) needs no transposition of the big matrix;
  only g (tiny) is PE-transposed. Softmax max-subtraction is skipped
  (|scores| ~ 30, exp stays comfortably in fp32 range).
- exp(sT) is written in bf16; the softmax denominator (a cross-partition
  sum) is tree-reduced over the 8 m-tiles on DVE (bf16 4x mode), then a
  single ones-[128,128] matmul per n-half contracts the partitions and
  broadcasts the result -- 1/8th the PE cost of matmul-ing all 8 m-tiles.
- Attend runs bf16 (gT stationary, exp moving); scores/convs/w_o stay
  float32r (1 cycle/row at free>=256).
- theta PSUM->SBUF staging copies run on GPSIMD; phi/g maxpools read conv
  PSUM directly (no staging copy); the final gamma*o+x scale-add runs on
  GPSIMD -- keeps ACT (exp) and DVE under the PE roofline.
- Software pipelining: iteration q emits scores+exp for quarter q
  interleaved per-m-tile with attend/denom/w_o/output for quarter q-1;
  the next sample's convs+pools are emitted inside the current sample's
  last (attend-only) iteration, and its x is prefetched mid-loop.
- PSUM plan: tag "sT" [128,2,512] x2 bufs + tag "oud" [128,2,512] x2
  bufs = exactly 8 banks; w_o output reuses the freed oud tile.
"""
import sys

sys.path.insert(0, '/opt/trn_rl_repo')

from contextlib import ExitStack

import numpy as np

import concourse.bass as bass
import concourse.tile as tile
from concourse import bacc, mybir
from concourse.bass_utils import run_bass_kernel_spmd
from concourse.masks import make_identity

F32 = mybir.dt.float32
F32R = mybir.dt.float32r
BF16 = mybir.dt.bfloat16
AF = mybir.ActivationFunctionType
OP = mybir.AluOpType

B, C, H, W = 16, 256, 64, 64
HW = H * W            # 4096
M_POOL = HW // 4      # 1024
NCORES = 8
BPC = B // NCORES     # samples per core = 2


def build_kernel(nc, tc, ctx, x_d, wt_d, wp_d, wg_d, wo_d, gamma_d, out_d):
    sb = ctx.enter_context(tc.tile_pool(name="sb", bufs=1))
    per_s = ctx.enter_context(tc.tile_pool(name="per_s", bufs=2))
    stage1 = ctx.enter_context(tc.tile_pool(name="stage1", bufs=1))
    expp = ctx.enter_context(tc.tile_pool(name="expp", bufs=2))
    treep = ctx.enter_context(tc.tile_pool(name="treep", bufs=2))
    outp = ctx.enter_context(tc.tile_pool(name="outp", bufs=2))
    xp = ctx.enter_context(tc.tile_pool(name="xp", bufs=1))
    # PSUM: tag "sT" = [128,2,512] (2 banks) x 2 bufs; tag "oud" = [128,2,512]
    # (2 banks) x 2 bufs -> exactly 8 banks.
    big = ctx.enter_context(tc.tile_pool(name="big", bufs=1, space="PSUM"))

    def load_x(b, eng, start=0):
        qs = []
        for qq in range(start, 4):
            x_t = xp.tile([128, 2, 1024], F32R, name="x_t", bufs=5)
            eng.dma_start(
                x_t[:],
                x_d[b].rearrange("(c2 p) n -> p c2 n", p=128)[:, :, 1024 * qq:1024 * qq + 1024].bitcast(F32R),
            )
            qs.append(x_t)
        return qs

    # ---- constants (ordered to unblock convs ASAP: x first, it's the
    # long pole) ----
    x_q0 = xp.tile([128, 2, 1024], F32R, name="x_t", bufs=5)
    for _hx in range(2):
        nc.sync.dma_start(
            x_q0[:, :, 512 * _hx:512 * _hx + 512],
            x_d[0].rearrange("(c2 p) n -> p c2 n", p=128)[:, :, 512 * _hx:512 * _hx + 512].bitcast(F32R),
        )
    wtp_nat = sb.tile([64, 256], F32R)
    nc.sync.dma_start(wtp_nat[0:32, :], wt_d.bitcast(F32R))
    nc.sync.dma_start(wtp_nat[32:64, :], wp_d.bitcast(F32R))
    wg_nat = sb.tile([128, 256], F32R)
    nc.sync.dma_start(wg_nat[:], wg_d.bitcast(F32R))

    ident_f = sb.tile([128, 128], F32)
    make_identity(nc, ident_f[:])
    ident = sb.tile([128, 128], F32R)
    nc.vector.tensor_copy(ident[:], ident_f[:])

    x_qs_next = [x_q0] + load_x(0, nc.sync, start=1)

    wo_nat = sb.tile([128, 2, 128], F32R)
    nc.sync.dma_start(
        wo_nat[:], wo_d.rearrange("(two p) c -> p two c", p=128).bitcast(F32R)
    )
    gamma_bc = sb.tile([128, 1], F32)
    nc.sync.dma_start(gamma_bc[:], gamma_d.to_broadcast((128, 1)))

    wtp = sb.tile([128, 2, 64], F32R)     # [c_in_chunk, chunk, 64=theta|phi]
    wg = sb.tile([128, 2, 128], F32R)     # [c_in_chunk, chunk, 128 g-ch]
    for cc in range(2):
        trp_ps = big.tile([128, 2, 512], F32R, name="trp_ps", tag="sT", bufs=3)
        nc.tensor.transpose(
            trp_ps[:, 0, 0:64], wtp_nat[:, 128 * cc:128 * cc + 128], ident[0:64, 0:64]
        )
        nc.vector.tensor_copy(wtp[:, cc, :], trp_ps[:, 0, 0:64])
    for cc in range(2):
        trw_ps = big.tile([128, 2, 512], F32R, name="trw_ps", tag="sT", bufs=3)
        nc.tensor.transpose(
            trw_ps[:, 0, 0:128], wg_nat[:, 128 * cc:128 * cc + 128], ident[:]
        )
        nc.vector.tensor_copy(wg[:, cc, :], trw_ps[:, 0, 0:128])

    ident_bf = sb.tile([128, 128], BF16)
    nc.vector.tensor_copy(ident_bf[:], ident_f[:])
    ones_bf = sb.tile([128, 128], BF16)
    nc.vector.memset(ones_bf[:], 1.0)

    woT = sb.tile([128, 2, 128], F32R)    # [c(128), half, oc(128)]
    wo_pending = [True]

    def emit_wo_transposes():
        if not wo_pending[0]:
            return
        wo_pending[0] = False
        for cc in range(2):
            trg_ps = big.tile([128, 2, 512], F32R, name="trg_ps", tag="sT", bufs=3)
            nc.tensor.transpose(trg_ps[:, 0, 0:128], wo_nat[:, cc, :], ident[:])
            nc.vector.tensor_copy(woT[:, cc, :], trg_ps[:, 0, 0:128])

    def make_state():
        st = {}
        st["theta_q"] = []
        phi = per_s.tile([32, M_POOL], F32R, name="phi")
        g_sb = per_s.tile([128, M_POOL], F32R, name="g_sb")
        gT = per_s.tile([128, 8, 128], BF16, name="gT", bufs=1)
        st["phi"], st["g_sb"], st["gT"] = phi, g_sb, gT
        return st

    def emit_gtr(st, c4):
        g_sb, gT = st["g_sb"], st["gT"]
        for t in (2 * c4, 2 * c4 + 1):
            gtr_ps = big.tile([128, 2, 512], F32R, name="gtr_ps", tag="sT", bufs=3)
            nc.tensor.transpose(
                gtr_ps[:, 0, 0:128], g_sb[:, 128 * t:128 * t + 128], ident[:]
            )
            nc.scalar.copy(gT[:, t, :], gtr_ps[:, 0, 0:128].bitcast(F32))

    def emit_tp_chunk(st, x_qs_b, c4):
        phi = st["phi"]
        tp_ps = big.tile([64, 2, 512], F32, name="tp_ps", tag="sT", bufs=3)
        for sub in range(2):
            xoff = 512 * sub
            nc.tensor.matmul(
                tp_ps[:, sub, :], wtp[:, 0, :], x_qs_b[c4][:, 0, xoff:xoff + 512],
                start=True, stop=False,
            )
            nc.tensor.matmul(
                tp_ps[:, sub, :], wtp[:, 1, :], x_qs_b[c4][:, 1, xoff:xoff + 512],
                start=False, stop=True,
            )
        tpf = stage1.tile([32, 1024], F32R, name="tpf", bufs=4)
        nc.scalar.copy(tpf[:], tp_ps[0:32, :, :])
        st["theta_q"].append(tpf)
        # fused 2x2 maxpool: one tensor_reduce per tensor per chunk, reading
        # conv PSUM directly (rows h=(i2 tp), cols w=(w2 t); reduce (tp,t))
        pfv = tp_ps[32:64, :, :].rearrange(
            "p a (i2 tp w2 t) -> p a i2 w2 tp t", i2=4, tp=2, w2=32, t=2)
        nc.vector.tensor_reduce(
            phi[:, 256 * c4:256 * c4 + 256].rearrange(
                "p (a i2 w2) -> p a i2 w2", a=2, i2=4),
            pfv, axis=mybir.AxisListType.XY, op=OP.max,
        )

    def emit_g_chunk(st, x_qs_b, c4):
        g_sb = st["g_sb"]
        g_ps = big.tile([128, 2, 512], F32, name="g_ps", tag="sT", bufs=3)
        for sub in range(2):
            xoff = 512 * sub
            nc.tensor.matmul(
                g_ps[:, sub, :], wg[:, 0, :], x_qs_b[c4][:, 0, xoff:xoff + 512],
                start=True, stop=False,
            )
            nc.tensor.matmul(
                g_ps[:, sub, :], wg[:, 1, :], x_qs_b[c4][:, 1, xoff:xoff + 512],
                start=False, stop=True,
            )
        gfv = g_ps[:, :, :].rearrange(
            "p a (i2 tp w2 t) -> p a i2 w2 tp t", i2=4, tp=2, w2=32, t=2)
        nc.vector.tensor_reduce(
            g_sb[:, 256 * c4:256 * c4 + 256].rearrange(
                "p (a i2 w2) -> p a i2 w2", a=2, i2=4),
            gfv, axis=mybir.AxisListType.XY, op=OP.max,
        )
        emit_gtr(st, c4)

    st = make_state()
    emit_wo_transposes()
    for c4 in range(4):
        emit_tp_chunk(st, x_qs_next, c4)
    st_next = None
    for b in range(BPC):
        x_qs = x_qs_next
        theta_q, phi, gT = st["theta_q"], st["phi"], st["gT"]
        emit_wo_transposes()
        # ---- main loop over n-quarters, software pipelined ----
        # iteration q: scores+exp for quarter q (q<4) interleaved per m-tile
        # with attend/denom/wo/out for quarter q-1 (q>=1).
        expST_prev = None
        partl_prev = None
        partr_prev = None
        tr = {}
        for q in range(5):
            expST = None
            if q < 4:
                expST = expp.tile([128, 8, 1024], BF16, name="expST")
            out_q = None
            oud = {}
            wops = {}
            if q == 2 and b + 1 < BPC:
                x_qs_next = load_x(b + 1, nc.sync)
            for u in range(8):  # per m-tile unit
                if q >= 1:
                    sub, seg = u // 4, u % 4  # seg: 2 m-tiles each
                    if seg == 0:
                        oud[sub] = big.tile([128, 512], F32, name="att", tag="oud", bufs=2)
                    if seg == 2:
                        # softmax denominator for this n-half: single matmul
                        # on the tree-reduced partial, broadcast over parts.
                        # It parks in wo_ps[:,0,:]; the w_o matmul overwrites
                        # after recip consumed it.
                        wops[sub] = big.tile([128, 2, 512], F32, name="wo_ps", tag="sT", bufs=3)
                        nc.tensor.matmul(
                            wops[sub][:, 0, :], ones_bf[:],
                            partl_prev[:, 512 * sub:512 * sub + 512],
                            start=True, stop=False,
                        )
                        nc.tensor.matmul(
                            wops[sub][:, 0, :], ones_bf[:],
                            partr_prev[:, 512 * sub:512 * sub + 512],
                            start=False, stop=True,
                        )
                    for tl in range(2):
                        t = 2 * seg + tl
                        nc.tensor.matmul(
                            oud[sub][:, :], gT[:, t, :],
                            expST_prev[:, t, 512 * sub:512 * sub + 512],
                            start=(t == 0), stop=(t == 7),
                        )
                    if seg == 3:
                        if out_q is None:
                            out_q = outp.tile([128, 2, 1024], F32, name="out_q")
                        recip = outp.tile([128, 512], F32, name="recip")
                        oUr = outp.tile([128, 512], F32R, name="oUr")
                        wo_ps = wops[sub]
                        nc.vector.reciprocal_approx_fast(
                            out=recip[:], in_=wo_ps[:, 0, :]
                        )
                        nc.vector.scalar_tensor_tensor(
                            oUr[:], oud[sub][:, :], 1.0, recip[:],
                            op0=OP.mult, op1=OP.mult,
                        )
                        for h in range(2):
                            nc.tensor.matmul(
                                wo_ps[:, h, :], woT[:, h, :], oUr[:],
                                start=True, stop=True,
                            )
                        xoff = 512 * sub
                        for h in range(2):
                            nc.vector.scalar_tensor_tensor(
                                out_q[:, h, xoff:xoff + 512],
                                wo_ps[:, h, :], gamma_bc[:],
                                x_qs[q - 1][:, h, xoff:xoff + 512].bitcast(F32),
                                op0=OP.mult, op1=OP.add,
                            )
                        if b == BPC - 1 and q == 4:
                            nqp = 1024 * (q - 1)
                            for h in range(2):
                                nc.sync.dma_start(
                                    out_d[b, 128 * h:128 * h + 128, nqp + xoff:nqp + xoff + 512],
                                    out_q[:, h, xoff:xoff + 512],
                                )
                if q < 4:
                    sT_ps = big.tile([128, 2, 512], F32, name="sT_ps", tag="sT", bufs=3)
                    for sub in range(2):
                        nc.tensor.matmul(
                            sT_ps[:, sub, :],
                            phi[:, 128 * u:128 * u + 128],
                            theta_q[q][:, 512 * sub:512 * sub + 512],
                            start=True, stop=True,
                        )
                    nc.scalar.activation(
                        expST[:, u, :], sT_ps[:, :, :], AF.Exp
                    )
                    # denominator partial sums over m-tiles. L = u0..u3 built
                    # on GPSIMD (slow engine, but consumed a full iteration
                    # later); R = u4..u7 on DVE (short post-exp7 tail). The
                    # denominator matmul accumulates both moving tiles.
                    leng = nc.gpsimd if q < 3 else nc.vector
                    if u == 1:
                        t01 = treep.tile([128, 1024], BF16, name="t01")
                        leng.tensor_tensor(
                            t01[:], expST[:, 0, :], expST[:, 1, :], op=OP.add)
                        tr["t01"] = t01
                    if u == 3:
                        t23 = treep.tile([128, 1024], BF16, name="t23")
                        leng.tensor_tensor(
                            t23[:], expST[:, 2, :], expST[:, 3, :], op=OP.add)
                        part_l = treep.tile([128, 1024], BF16, name="part_l")
                        leng.tensor_tensor(
                            part_l[:], tr["t01"][:], t23[:], op=OP.add)
                    if u == 5:
                        t45 = treep.tile([128, 1024], BF16, name="t45")
                        nc.vector.tensor_tensor(
                            t45[:], expST[:, 4, :], expST[:, 5, :], op=OP.add)
                        tr["t45"] = t45
                    if u == 7:
                        t67 = treep.tile([128, 1024], BF16, name="t67")
                        nc.vector.tensor_tensor(
                            t67[:], expST[:, 6, :], expST[:, 7, :], op=OP.add)
                        part_r = treep.tile([128, 1024], BF16, name="part_r")
                        nc.vector.tensor_tensor(
                            part_r[:], tr["t45"][:], t67[:], op=OP.add)
                if q == 0 and u % 2 == 0:
                    emit_g_chunk(st, x_qs, u // 2)
                if q == 4 and b + 1 < BPC and u < 4:
                    if st_next is None:
                        st_next = make_state()
                    emit_tp_chunk(st_next, x_qs_next, u)
            if q >= 1 and not (b == BPC - 1 and q == 4):
                nqp = 1024 * (q - 1)
                for h in range(2):
                    nc.sync.dma_start(
                        out_d[b, 128 * h:128 * h + 128, nqp:nqp + 1024],
                        out_q[:, h, :],
                    )
            expST_prev = expST
            if q < 4:
                partl_prev = part_l
                partr_prev = part_r
        st = st_next
        st_next = None


_CACHE = {}


def _get_compiled():
    if "nc" in _CACHE:
        return _CACHE["nc"]
    nc = bacc.Bacc("TRN2", target_bir_lowering=False, debug=False,
                   num_devices=NCORES)
    x_d = nc.dram_tensor("x", [BPC, C, HW], F32, kind="ExternalInput").ap()
    wt_d = nc.dram_tensor("w_theta", [32, 256], F32, kind="ExternalInput").ap()
    wp_d = nc.dram_tensor("w_phi", [32, 256], F32, kind="ExternalInput").ap()
    wg_d = nc.dram_tensor("w_g", [128, 256], F32, kind="ExternalInput").ap()
    wo_d = nc.dram_tensor("w_o", [256, 128], F32, kind="ExternalInput").ap()
    gamma_d = nc.dram_tensor("gamma", [1, 1], F32, kind="ExternalInput").ap()
    out_d = nc.dram_tensor("out", [BPC, C, HW], F32, kind="ExternalOutput").ap()

    with tile.TileContext(nc) as tc:
        with ExitStack() as ctx:
            build_kernel(nc, tc, ctx, x_d, wt_d, wp_d, wg_d, wo_d, gamma_d,
                         out_d)
    nc.compile()
    _CACHE["nc"] = nc
    return nc


def kernel(x, w_theta, w_phi, w_g, w_o, gamma, _trace=False, _tmpdir=None):
    nc = _get_compiled()
    x = np.ascontiguousarray(np.asarray(x, dtype=np.float32))
    in_maps = []
    for c in range(NCORES):
        shard = x[c * BPC:(c + 1) * BPC].reshape(BPC, C, HW)
        in_maps.append({
            "x": np.ascontiguousarray(shard),
            "w_theta": np.asarray(w_theta, np.float32),
            "w_phi": np.asarray(w_phi, np.float32),
            "w_g": np.asarray(w_g, np.float32),
            "w_o": np.asarray(w_o, np.float32),
            "gamma": np.asarray(gamma, np.float32).reshape(1, 1),
        })
    kwargs = {}
    if _trace:
        kwargs = dict(trace=True, tmpdir=_tmpdir)
    res = run_bass_kernel_spmd(nc, in_maps, core_ids=list(range(NCORES)),
                               **kwargs)
    out = np.concatenate([r["out"] for r in res.results], axis=0)
    out = out.reshape(B, C, H, W).astype(np.float32)
    if _trace:
        return out, res
    return out
